# revision 16
# baseline (speedup 1.0000x reference)
"""3-layer GCN + gene-pair MLP on 8 Trainium2 NeuronCores (Bass/Tile).

v2 strategy (v1 in kernel_v1_backup.py)
---------------------------------------
Profiling v1 showed the kernel is bound by SWDGE descriptor generation on
the Pool engine (~8 ns/descriptor, ~520k descriptors) plus four ncfw
AllGathers.  v2 changes:
  * dma_gather calls rotate over 4 SWDGE queues (queues 1-3 run async on
    separate Q7 workers -> ~2.5 ns/desc aggregate).
  * The layer-1 table is just x (bf16) replicated to every core as an
    input, with W1 applied AFTER aggregation (A@(xW) == (A@x)W), removing
    one AllGather + the x@W1 staging pass entirely.
  * All layers post-multiply their weight; the uv table folds W3@Wfc1
    (and every bias) on the host, so layer tables carry raw aggregates.
  * The one-hot scatter matrices S (GCN edge weights folded in) are
    host-precomputed and DMA-loaded per layer instead of built with two
    DVE passes per chunk each layer.
  * Bias folding uses a 65-row contraction (ones row appended to the
    aggregate tile, bias row appended to the weight).
Aggregation itself keeps v1's scheme: edges sorted by dst tile, gathered
hw[src] rows (256B each) reduced per 128-edge chunk with one matmul
acc[64,V] += G^T @ S into PSUM.
"""
import sys
import os

sys.path.insert(0, "/opt/trn_rl_repo")

import numpy as np
import ml_dtypes

import concourse.bacc as bacc
import concourse.mybir as mybir
import concourse.tile as tile
from concourse.bass_utils import run_bass_kernel_spmd

bf16 = mybir.dt.bfloat16
f32 = mybir.dt.float32

R = int(os.environ.get("GCN_R", "8"))  # cores
V = 128          # nodes per aggregation tile
GT = int(os.environ.get("GCN_GT", "4"))  # tiles per gather group
MAXBAND = 30000  # int16-addressable rows per gather band (< 32768)
NQ = int(os.environ.get("GCN_NQ", "4"))  # SWDGE queues

_BF = ml_dtypes.bfloat16


def _ceil(a, b):
    return -(-a // b)


def _wrap_idx(flat):
    """dma_gather index layout: position j -> [j % 16, j // 16], x8 partitions."""
    n = len(flat)
    assert n % 128 == 0
    arr = np.ascontiguousarray(flat.reshape(n // 16, 16).T.astype(np.int16))
    return np.tile(arr, (8, 1))


class _Plan:
    pass


def _make_plan(x, src, dst, gene1, gene2):
    p = _Plan()
    N = x.shape[0]
    NP = gene1.shape[0]
    p.N, p.NP = N, NP
    p.NPR = _ceil(N, R)               # nodes per rank
    p.TPR = _ceil(p.NPR, 128)         # node tiles per rank
    p.ROWS_PR = p.TPR * 128           # table rows per rank
    p.TOT_ROWS = p.ROWS_PR * R
    p.NB = max(1, _ceil(p.TOT_ROWS, MAXBAND))
    p.BSZ = _ceil(p.TOT_ROWS, p.NB)   # rows per band (last may be short)
    assert p.BSZ < 32768
    p.NG = _ceil(p.TPR, GT)
    p.PPR = _ceil(NP, R)              # pairs per rank

    def row_of(n):
        r = n // p.NPR
        l = n - r * p.NPR
        return p.ROWS_PR * r + p.TPR * (l % 128) + (l // 128)

    p.row_of = row_of

    # ---- edge structure (shared across the 3 layers) ----
    own = (dst // p.NPR).astype(np.int64)
    loc = dst - own * p.NPR
    tl = loc // 128                     # tile within rank
    dl = (loc % 128).astype(np.int64)   # one-hot column
    rs = row_of(src)
    band = rs // p.BSZ
    ridx = (rs - band * p.BSZ).astype(np.int64)

    ones = np.ones(len(src), np.float32)
    out_deg = np.clip(np.bincount(src, weights=ones, minlength=N), 1.0, None)
    in_deg = np.clip(np.bincount(dst, weights=ones, minlength=N), 1.0, None)
    w = ((out_deg ** -0.5)[src] * (in_deg ** -0.5)[dst]).astype(np.float32)

    NBt = p.NB
    bid = (own * p.TPR + tl) * NBt + band
    counts = np.bincount(bid, minlength=R * p.TPR * NBt).reshape(R, p.TPR, NBt)
    Lmax = counts.max(axis=0)                      # [TPR, NB]
    p.Pch = _ceil(Lmax, 128)                       # chunks per (tile, band)

    # column/run offsets in (group, band, tile) order
    p.col_run = np.zeros((p.TPR, NBt), np.int64)
    p.gathers = []                                 # (g, b, col0, nch)
    col = 0
    for g in range(p.NG):
        ts = range(g * GT, min((g + 1) * GT, p.TPR))
        for b in range(NBt):
            c0 = col
            for t in ts:
                p.col_run[t, b] = col
                col += p.Pch[t, b]
            p.gathers.append((g, b, c0, col - c0))
    p.CT = int(col)
    E_pad = p.CT * 128

    # per-core flat slots
    order = np.argsort(bid, kind="stable")
    bid_s = bid[order]
    own_s = own[order]
    uniq, first = np.unique(bid_s, return_index=True)
    start_map = np.zeros(R * p.TPR * NBt, np.int64)
    start_map[uniq] = first
    i_within = np.arange(len(order)) - start_map[bid_s]
    # slot within the core's padded layout
    tl_s, band_s = tl[order], band[order]
    slot = p.col_run[tl_s, band_s] * 128 + i_within

    p.idx2 = np.zeros((R, 128, p.CT * 8), np.int16)
    p.S2 = np.zeros((R, 128, p.CT, 128), _BF)      # host-built scatter one-hots
    ridx_s, dl_ss, w_s = ridx[order], dl[order], w[order]
    for r in range(R):
        m = own_s == r
        idx_flat = np.zeros(E_pad, np.int64)
        idx_flat[slot[m]] = ridx_s[m]
        sl = slot[m]
        # S[e%128, e//128, dl] = w  (slot e)
        S = np.zeros((E_pad, 128), np.float32)
        S[sl, dl_ss[m]] = w_s[m]
        p.S2[r] = S.reshape(p.CT, 128, 128).transpose(1, 0, 2).astype(_BF)
        blocks = []
        for (_, _, c0, nch) in p.gathers:
            if nch == 0:
                continue
            blocks.append(_wrap_idx(idx_flat[c0 * 128:(c0 + nch) * 128]))
        p.idx2[r] = np.hstack(blocks)

    # ---- pair structure ----
    g1r, g2r = row_of(gene1), row_of(gene2)
    pb = (g1r // p.BSZ) * NBt + (g2r // p.BSZ)
    pown = np.arange(NP) // p.PPR
    NBK = NBt * NBt
    pcnt = np.bincount(pown * NBK + pb, minlength=R * NBK).reshape(R, NBK)
    Lp = pcnt.max(axis=0)
    p.Pchp = _ceil(Lp, 128)                        # chunks per bucket
    p.pcol = np.concatenate([[0], np.cumsum(p.Pchp)])
    p.PCT = int(p.pcol[-1])
    PP_pad = p.PCT * 128

    pbid = pown * NBK + pb
    porder = np.argsort(pbid, kind="stable")
    pbid_s = pbid[porder]
    pown_s = pown[porder]
    uq, fs = np.unique(pbid_s, return_index=True)
    smap = np.zeros(R * NBK, np.int64)
    smap[uq] = fs
    pi_within = np.arange(NP) - smap[pbid_s]
    pslot = p.pcol[pb[porder]] * 128 + pi_within

    p.pidx1 = np.zeros((R, 128, p.PCT * 8), np.int16)
    p.pidx2 = np.zeros((R, 128, p.PCT * 8), np.int16)
    p.perm = np.full((R, PP_pad), -1, np.int64)
    r1 = (g1r - (g1r // p.BSZ) * p.BSZ)[porder]
    r2 = (g2r - (g2r // p.BSZ) * p.BSZ)[porder]
    for r in range(R):
        m = pown_s == r
        f1 = np.zeros(PP_pad, np.int64)
        f2 = np.zeros(PP_pad, np.int64)
        f1[pslot[m]] = r1[m]
        f2[pslot[m]] = r2[m]
        p.perm[r][pslot[m]] = porder[m]
        b1s, b2s = [], []
        for bkt in range(NBK):
            c0, nch = p.pcol[bkt], p.Pchp[bkt]
            if nch == 0:
                continue
            b1s.append(_wrap_idx(f1[c0 * 128:(c0 + nch) * 128]))
            b2s.append(_wrap_idx(f2[c0 * 128:(c0 + nch) * 128]))
        p.pidx1[r] = np.hstack(b1s)
        p.pidx2[r] = np.hstack(b2s)
    return p


def _build(p):
    """Build the SPMD Bass program for plan `p`."""
    nc = bacc.Bacc("TRN2", num_devices=R, num_swdge_queues=NQ)
    NBt, NBK = p.NB, p.NB * p.NB

    xtab_d = nc.dram_tensor("xtab", [p.TOT_ROWS, 128], bf16, kind="ExternalInput")
    S_d = nc.dram_tensor("Smat", [128, p.CT, 128], bf16, kind="ExternalInput")
    idx_d = nc.dram_tensor("idxE", [128, p.CT * 8], mybir.dt.int16, kind="ExternalInput")
    pi1_d = nc.dram_tensor("pidx1", [128, p.PCT * 8], mybir.dt.int16, kind="ExternalInput")
    pi2_d = nc.dram_tensor("pidx2", [128, p.PCT * 8], mybir.dt.int16, kind="ExternalInput")
    Ws_d = nc.dram_tensor("Ws", [65, 4, 64], bf16, kind="ExternalInput")
    wdbd_d = nc.dram_tensor("wdbd", [128, 65], f32, kind="ExternalInput")
    pout_d = nc.dram_tensor("pout", [128, p.PCT, 2], f32, kind="ExternalOutput")

    rg = [list(range(R))]
    qrr = [0]

    def next_q():
        # queues 1..3 only: their desc-gen runs async on separate Q7
        # workers; a queue-0 call executes inline and blocks the Pool
        # queue ~12us, which paces the whole gather stream.
        q = 1 + qrr[0] % (NQ - 1)
        qrr[0] += 1
        return q

    with tile.TileContext(nc) as tc:
        with tc.tile_pool(name="dloc", bufs=1, space="DRAM") as dloc, \
             tc.tile_pool(name="sb", bufs=1) as sb, \
             tc.tile_pool(name="ps", bufs=1, space="PSUM") as ps:

            stage_dram = dloc.tile([128, p.TPR, 128], bf16)
            fulls = [dloc.tile([p.TOT_ROWS, 128], bf16, tag=f"full{i}",
                               name=f"full{i}", addr_space="Shared")
                     for i in range(3)]

            idx_t = sb.tile([128, p.CT * 8], mybir.dt.int16)
            pi1_t = sb.tile([128, p.PCT * 8], mybir.dt.int16)
            pi2_t = sb.tile([128, p.PCT * 8], mybir.dt.int16)
            Ws_t = sb.tile([65, 4, 64], bf16)
            wdbd_t = sb.tile([128, 65], f32)
            for t_, d_ in ((idx_t, idx_d), (pi1_t, pi1_d), (pi2_t, pi2_d),
                           (Ws_t, Ws_d), (wdbd_t, wdbd_d)):
                nc.sync.dma_start(out=t_[:], in_=d_[:])

            # two explicit acc buffers with a ones row at partition 64
            accs = [sb.tile([65, 128], bf16, tag=f"accsb{i}", name=f"accsb{i}")
                    for i in range(4)]
            for a in accs:
                nc.vector.memset(a[64:65, :], 1.0)

            stage_sb = sb.tile([128, p.TPR, 128], bf16)

            for l in range(3):
                table = xtab_d if l == 0 else fulls[l - 1]
                for g in range(p.NG):
                    ts = range(g * GT, min((g + 1) * GT, p.TPR))
                    Gs, Ss, c0s = {}, {}, {}
                    for (gg, b, c0, nch) in p.gathers:
                        if gg != g or nch == 0:
                            continue
                        c0s[b] = c0
                        Gt = sb.tile([128, nch, 128], bf16, tag="G", bufs=10)
                        lo = b * p.BSZ
                        hi = min(lo + p.BSZ, p.TOT_ROWS)
                        nc.gpsimd.dma_gather(
                            out_ap=Gt[:], in_ap=table[lo:hi, :],
                            idxs_ap=idx_t[:, c0 * 8:(c0 + nch) * 8],
                            num_idxs=nch * 128, num_idxs_reg=nch * 128,
                            elem_size=128, single_packet=False,
                            queue_num=next_q())
                        St = sb.tile([128, nch, 128], bf16, tag="S", bufs=10)
                        nc.scalar.dma_start(out=St[:], in_=S_d[:, c0:c0 + nch, :])
                        Gs[b], Ss[b] = Gt, St
                    g_lo, g_hi = g * GT, min((g + 1) * GT, p.TPR)
                    for t in ts:
                        nch_t = int(p.Pch[t, :].sum())
                        if nch_t == 0:
                            continue
                        acc = ps.tile([64, V], f32, tag="acc", space="PSUM", bufs=2)
                        ki = 0
                        for b in range(NBt):
                            base = int(p.col_run[t, b] - c0s.get(b, 0))
                            for k in range(int(p.Pch[t, b])):
                                nc.tensor.matmul(
                                    out=acc[:],
                                    lhsT=Gs[b][:, base + k, 0:64],
                                    rhs=Ss[b][:, base + k, :],
                                    start=(ki == 0), stop=(ki == nch_t - 1))
                                ki += 1
                        a_sb = accs[t % 4]
                        nc.vector.tensor_copy(a_sb[0:64, :], acc[:])
                        if l < 2:
                            hp = ps.tile([128, 64], f32, tag="hp", space="PSUM",
                                         bufs=6)
                            nc.tensor.matmul(out=hp[:], lhsT=a_sb[:, :],
                                             rhs=Ws_t[:, l, :], start=True,
                                             stop=True)
                            nc.vector.tensor_scalar(
                                out=stage_sb[:, t, 0:64], in0=hp[:],
                                scalar1=0.0, scalar2=None,
                                op0=mybir.AluOpType.max)
                        else:
                            up = ps.tile([128, 64], f32, tag="hp", space="PSUM",
                                         bufs=6)
                            vp = ps.tile([128, 64], f32, tag="hp", space="PSUM",
                                         bufs=6)
                            nc.tensor.matmul(out=up[:], lhsT=a_sb[:, :],
                                             rhs=Ws_t[:, 2, :], start=True,
                                             stop=True)
                            nc.tensor.matmul(out=vp[:], lhsT=a_sb[:, :],
                                             rhs=Ws_t[:, 3, :], start=True,
                                             stop=True)
                            nc.vector.tensor_copy(stage_sb[:, t, 0:64], up[:])
                            nc.vector.tensor_copy(stage_sb[:, t, 64:128], vp[:])
                    nc.sync.dma_start(out=stage_dram[:, g_lo:g_hi, :],
                                      in_=stage_sb[:, g_lo:g_hi, :])
                if l < 2:
                    nc.gpsimd.collective_compute(
                        "AllGather", mybir.AluOpType.bypass, replica_groups=rg,
                        ins=[stage_dram[:]], outs=[fulls[l][:]])

            # ---- pair stage ----
            # u-gathers read the local uv stage (stage_dram rows) and are
            # emitted BEFORE the final AllGather so they overlap it;
            # v-gathers read the gathered table per band afterwards.
            uvf = fulls[2]
            stage_rows = stage_dram.tensor.reshape([p.ROWS_PR, 128])
            SUBB = 12
            blocks = []
            for bkt in range(NBt):
                c0, nch = int(p.pcol[bkt]), int(p.Pchp[bkt])
                for s0 in range(0, nch, SUBB):
                    blocks.append((bkt, c0 + s0, min(SUBB, nch - s0)))
            NBLK = len(blocks)
            Uts = []
            for (bkt, c0, nch) in blocks:
                Ut = sb.tile([128, nch, 128], bf16, tag="U", bufs=NBLK)
                nc.gpsimd.dma_gather(
                    out_ap=Ut[:], in_ap=stage_rows[:, :],
                    idxs_ap=pi1_t[:, c0 * 8:(c0 + nch) * 8],
                    num_idxs=nch * 128, num_idxs_reg=nch * 128,
                    elem_size=128, single_packet=False,
                    queue_num=next_q())
                Uts.append(Ut)
            nc.gpsimd.collective_compute(
                "AllGather", mybir.AluOpType.bypass, replica_groups=rg,
                ins=[stage_dram[:]], outs=[fulls[2][:]])
            for (bkt, c0, nch), Ut in zip(blocks, Uts):
                Vt = sb.tile([128, nch, 128], bf16, tag="Vt", bufs=6)
                lo = bkt * p.BSZ
                hi = min(lo + p.BSZ, p.TOT_ROWS)
                nc.gpsimd.dma_gather(
                    out_ap=Vt[:], in_ap=uvf[lo:hi, :],
                    idxs_ap=pi2_t[:, c0 * 8:(c0 + nch) * 8],
                    num_idxs=nch * 128, num_idxs_reg=nch * 128,
                    elem_size=128, single_packet=False,
                    queue_num=next_q())
                z = sb.tile([128, nch, 64], f32, tag="z", bufs=2)
                nc.vector.tensor_tensor(out=z[:], in0=Ut[:, :, 0:64],
                                        in1=Vt[:, :, 64:128],
                                        op=mybir.AluOpType.add)
                nc.vector.tensor_scalar_max(z[:], z[:], 0.0)
                nc.vector.tensor_tensor(
                    out=z[:], in0=z[:],
                    in1=wdbd_t[:, 0:64].unsqueeze(1).to_broadcast([128, nch, 64]),
                    op=mybir.AluOpType.mult)
                ds = sb.tile([128, nch], f32, tag="ds", bufs=2)
                nc.vector.tensor_reduce(out=ds[:], in_=z[:],
                                        axis=mybir.AxisListType.X,
                                        op=mybir.AluOpType.add)
                po = sb.tile([128, nch, 2], f32, tag="po", bufs=2)
                nc.scalar.activation(po[:, :, 1:2], ds[:].unsqueeze(2),
                                     mybir.ActivationFunctionType.Sigmoid,
                                     bias=wdbd_t[:, 64:65], scale=1.0)
                nc.vector.tensor_scalar(
                    out=po[:, :, 0:1], in0=po[:, :, 1:2],
                    scalar1=-1.0, scalar2=1.0,
                    op0=mybir.AluOpType.mult, op1=mybir.AluOpType.add)
                nc.sync.dma_start(out=pout_d[:, c0:c0 + nch, :], in_=po[:])
    nc.compile()
    return nc


def _split_excess_waits(nc, max_waits=1):
    """Walrus rejects >1 sem wait on queue instructions; hoist extras onto
    standalone EventSemaphore instructions placed just before."""
    for fn in nc.m.functions:
        for bb in fn.blocks:
            il = bb.instructions
            new_list = []
            changed = False
            for ins in il:
                si = ins.sync_info
                if si is not None and si.on_wait and len(si.on_wait) > max_waits:
                    waits = list(si.on_wait)
                    keep, excess = waits[:max_waits], waits[max_waits:]
                    for gi in range(0, len(excess), max_waits):
                        ev = mybir.InstEventSemaphore(
                            name=f"{ins.name}_wsplit{gi}", ins=[], outs=[])
                        ev.engine = ins.engine
                        ev.sync_info = mybir.SyncInfo(
                            on_wait=excess[gi:gi + max_waits], on_update=[])
                        new_list.append(ev)
                    ins.sync_info = mybir.SyncInfo(
                        on_wait=keep, on_update=list(si.on_update))
                    changed = True
                new_list.append(ins)
            if changed:
                bb.instructions = new_list


def kernel(x, src, dst, gene1, gene2, W1, b1, W2, b2, W3, b3,
           Wfc1, bfc1, Wfc2, bfc2, _trace=False):
    x = np.asarray(x, np.float32)
    src = np.asarray(src, np.int64)
    dst = np.asarray(dst, np.int64)
    gene1 = np.asarray(gene1, np.int64)
    gene2 = np.asarray(gene2, np.int64)
    W1, b1 = np.asarray(W1, np.float32), np.asarray(b1, np.float32)
    W2, b2 = np.asarray(W2, np.float32), np.asarray(b2, np.float32)
    W3, b3 = np.asarray(W3, np.float32), np.asarray(b3, np.float32)
    Wfc1, bfc1 = np.asarray(Wfc1, np.float32), np.asarray(bfc1, np.float32)
    Wfc2, bfc2 = np.asarray(Wfc2, np.float32), np.asarray(bfc2, np.float32)

    p = _make_plan(x, src, dst, gene1, gene2)

    # host-folded constants: Ws rows 0:64 = weight, row 64 = bias
    Mu = W3 @ Wfc1[:64]
    Mv = W3 @ Wfc1[64:]
    bu = b3 @ Wfc1[:64] + 0.5 * bfc1
    bv = b3 @ Wfc1[64:] + 0.5 * bfc1
    Ws = np.zeros((65, 4, 64), np.float32)
    Ws[:64, 0], Ws[64, 0] = W1, b1
    Ws[:64, 1], Ws[64, 1] = W2, b2
    Ws[:64, 2], Ws[64, 2] = Mu, bu
    Ws[:64, 3], Ws[64, 3] = Mv, bv
    wdiff = (Wfc2[:, 1] - Wfc2[:, 0]).astype(np.float32)
    bd = float(bfc2[1] - bfc2[0])
    wdbd = np.zeros((128, 65), np.float32)
    wdbd[:, 0:64] = wdiff[None, :]
    wdbd[:, 64] = bd

    nc = _build(p)
    if not os.environ.get("GCN_SIM"):
        _split_excess_waits(nc)

    # replicated x table (layer-1 gather source), bf16, baseline row layout
    xtab = np.zeros((p.TOT_ROWS, 128), _BF)
    xb = x.astype(_BF)
    for r in range(R):
        lo = r * p.NPR
        hi = min(lo + p.NPR, p.N)
        blk = np.zeros((p.ROWS_PR, 64), _BF)
        blk[:hi - lo] = xb[lo:hi]
        # local node l -> row TPR*(l%128) + l//128
        blk = blk.reshape(p.TPR, 128, 64).transpose(1, 0, 2).reshape(p.ROWS_PR, 64)
        xtab[r * p.ROWS_PR:(r + 1) * p.ROWS_PR, 0:64] = blk

    in_maps = []
    for r in range(R):
        m = {
            "xtab": xtab,
            "Smat": p.S2[r].reshape(128, p.CT, 128),
            "idxE": p.idx2[r],
            "pidx1": p.pidx1[r], "pidx2": p.pidx2[r],
            "Ws": Ws.astype(_BF), "wdbd": wdbd,
        }
        in_maps.append(m)

    if os.environ.get("GCN_SIM"):
        from concourse.bass_interp import MultiCoreSim
        sim = MultiCoreSim(nc, R)
        for r in range(R):
            for k, v in in_maps[r].items():
                sim.cores[r].tensor(k)[:] = v
        sim.simulate()
        results = [{"pout": np.asarray(sim.cores[rr].mem_tensor("pout"))
                    .reshape(128, p.PCT, 2)} for rr in range(R)]

        class _R:
            pass
        res = _R()
        res.results = results
    else:
        res = run_bass_kernel_spmd(nc, in_maps, core_ids=list(range(R)),
                                   trace=_trace)

    out = np.zeros((p.NP, 2), np.float32)
    for r in range(R):
        po = np.asarray(res.results[r]["pout"]).reshape(128, p.PCT, 2)
        flat = po.transpose(1, 0, 2).reshape(-1, 2)   # slot j = c*128 + p
        valid = p.perm[r] >= 0
        out[p.perm[r][valid]] = flat[valid]
    if _trace:
        kernel.last_results = res
    return out


# revision 17
# speedup vs baseline: 1.0510x; 1.0510x over previous
"""3-layer GCN + gene-pair MLP on 8 Trainium2 NeuronCores (Bass/Tile).

v2 strategy (v1 in kernel_v1_backup.py)
---------------------------------------
Profiling v1 showed the kernel is bound by SWDGE descriptor generation on
the Pool engine (~8 ns/descriptor, ~520k descriptors) plus four ncfw
AllGathers.  v2 changes:
  * dma_gather calls rotate over 4 SWDGE queues (queues 1-3 run async on
    separate Q7 workers -> ~2.5 ns/desc aggregate).
  * The layer-1 table is just x (bf16) replicated to every core as an
    input, with W1 applied AFTER aggregation (A@(xW) == (A@x)W), removing
    one AllGather + the x@W1 staging pass entirely.
  * All layers post-multiply their weight; the uv table folds W3@Wfc1
    (and every bias) on the host, so layer tables carry raw aggregates.
  * The one-hot scatter matrices S (GCN edge weights folded in) are
    host-precomputed and DMA-loaded per layer instead of built with two
    DVE passes per chunk each layer.
  * Bias folding uses a 65-row contraction (ones row appended to the
    aggregate tile, bias row appended to the weight).
Aggregation itself keeps v1's scheme: edges sorted by dst tile, gathered
hw[src] rows (256B each) reduced per 128-edge chunk with one matmul
acc[64,V] += G^T @ S into PSUM.
"""
import sys
import os

sys.path.insert(0, "/opt/trn_rl_repo")

import numpy as np
import ml_dtypes

import concourse.bacc as bacc
import concourse.mybir as mybir
import concourse.tile as tile
from concourse.bass_utils import run_bass_kernel_spmd

bf16 = mybir.dt.bfloat16
f32 = mybir.dt.float32

R = int(os.environ.get("GCN_R", "8"))  # cores
V = 128          # nodes per aggregation tile
GT = int(os.environ.get("GCN_GT", "4"))  # tiles per gather group
MAXBAND = 30000  # int16-addressable rows per gather band (< 32768)
NQ = int(os.environ.get("GCN_NQ", "4"))  # SWDGE queues

_BF = ml_dtypes.bfloat16


def _ceil(a, b):
    return -(-a // b)


def _wrap_idx(flat):
    """dma_gather index layout: position j -> [j % 16, j // 16], x8 partitions."""
    n = len(flat)
    assert n % 128 == 0
    arr = np.ascontiguousarray(flat.reshape(n // 16, 16).T.astype(np.int16))
    return np.tile(arr, (8, 1))


class _Plan:
    pass


def _make_plan(x, src, dst, gene1, gene2):
    p = _Plan()
    N = x.shape[0]
    NP = gene1.shape[0]
    p.N, p.NP = N, NP
    p.NPR = _ceil(N, R)               # nodes per rank
    p.TPR = _ceil(p.NPR, 128)         # node tiles per rank
    p.ROWS_PR = p.TPR * 128           # table rows per rank
    p.TOT_ROWS = p.ROWS_PR * R
    p.NB = max(1, _ceil(p.TOT_ROWS, MAXBAND))
    p.BSZ = _ceil(p.TOT_ROWS, p.NB)   # rows per band (last may be short)
    assert p.BSZ < 32768
    p.NG = _ceil(p.TPR, GT)
    p.PPR = _ceil(NP, R)              # pairs per rank

    def row_of(n):
        r = n // p.NPR
        l = n - r * p.NPR
        return p.ROWS_PR * r + p.TPR * (l % 128) + (l // 128)

    p.row_of = row_of

    # ---- edge structure (shared across the 3 layers) ----
    own = (dst // p.NPR).astype(np.int64)
    loc = dst - own * p.NPR
    tl = loc // 128                     # tile within rank
    dl = (loc % 128).astype(np.int64)   # one-hot column
    rs = row_of(src)
    band = rs // p.BSZ
    ridx = (rs - band * p.BSZ).astype(np.int64)

    ones = np.ones(len(src), np.float32)
    out_deg = np.clip(np.bincount(src, weights=ones, minlength=N), 1.0, None)
    in_deg = np.clip(np.bincount(dst, weights=ones, minlength=N), 1.0, None)
    w = ((out_deg ** -0.5)[src] * (in_deg ** -0.5)[dst]).astype(np.float32)

    NBt = p.NB
    bid = (own * p.TPR + tl) * NBt + band
    counts = np.bincount(bid, minlength=R * p.TPR * NBt).reshape(R, p.TPR, NBt)
    Lmax = counts.max(axis=0)                      # [TPR, NB]
    p.Pch = _ceil(Lmax, 128)                       # chunks per (tile, band)

    # column/run offsets in (group, band, tile) order
    p.col_run = np.zeros((p.TPR, NBt), np.int64)
    p.gathers = []                                 # (g, b, col0, nch)
    col = 0
    for g in range(p.NG):
        ts = range(g * GT, min((g + 1) * GT, p.TPR))
        for b in range(NBt):
            c0 = col
            for t in ts:
                p.col_run[t, b] = col
                col += p.Pch[t, b]
            p.gathers.append((g, b, c0, col - c0))
    p.CT = int(col)
    E_pad = p.CT * 128

    # per-core flat slots
    order = np.argsort(bid, kind="stable")
    bid_s = bid[order]
    own_s = own[order]
    uniq, first = np.unique(bid_s, return_index=True)
    start_map = np.zeros(R * p.TPR * NBt, np.int64)
    start_map[uniq] = first
    i_within = np.arange(len(order)) - start_map[bid_s]
    # slot within the core's padded layout
    tl_s, band_s = tl[order], band[order]
    slot = p.col_run[tl_s, band_s] * 128 + i_within

    p.idx2 = np.zeros((R, 128, p.CT * 8), np.int16)
    p.S2 = np.zeros((R, 128, p.CT, 128), _BF)      # host-built scatter one-hots
    ridx_s, dl_ss, w_s = ridx[order], dl[order], w[order]
    for r in range(R):
        m = own_s == r
        idx_flat = np.zeros(E_pad, np.int64)
        idx_flat[slot[m]] = ridx_s[m]
        sl = slot[m]
        # S[e%128, e//128, dl] = w  (slot e)
        S = np.zeros((E_pad, 128), np.float32)
        S[sl, dl_ss[m]] = w_s[m]
        p.S2[r] = S.reshape(p.CT, 128, 128).transpose(1, 0, 2).astype(_BF)
        blocks = []
        for (_, _, c0, nch) in p.gathers:
            if nch == 0:
                continue
            blocks.append(_wrap_idx(idx_flat[c0 * 128:(c0 + nch) * 128]))
        p.idx2[r] = np.hstack(blocks)

    # ---- pair structure ----
    g1r, g2r = row_of(gene1), row_of(gene2)
    pb = (g1r // p.BSZ) * NBt + (g2r // p.BSZ)
    pown = np.arange(NP) // p.PPR
    NBK = NBt * NBt
    pcnt = np.bincount(pown * NBK + pb, minlength=R * NBK).reshape(R, NBK)
    Lp = pcnt.max(axis=0)
    p.Pchp = _ceil(Lp, 128)                        # chunks per bucket
    p.pcol = np.concatenate([[0], np.cumsum(p.Pchp)])
    p.PCT = int(p.pcol[-1])
    PP_pad = p.PCT * 128

    pbid = pown * NBK + pb
    porder = np.argsort(pbid, kind="stable")
    pbid_s = pbid[porder]
    pown_s = pown[porder]
    uq, fs = np.unique(pbid_s, return_index=True)
    smap = np.zeros(R * NBK, np.int64)
    smap[uq] = fs
    pi_within = np.arange(NP) - smap[pbid_s]
    pslot = p.pcol[pb[porder]] * 128 + pi_within

    p.pidx1 = np.zeros((R, 128, p.PCT * 8), np.int16)
    p.pidx2 = np.zeros((R, 128, p.PCT * 8), np.int16)
    p.perm = np.full((R, PP_pad), -1, np.int64)
    r1 = (g1r - (g1r // p.BSZ) * p.BSZ)[porder]
    r2 = (g2r - (g2r // p.BSZ) * p.BSZ)[porder]
    for r in range(R):
        m = pown_s == r
        f1 = np.zeros(PP_pad, np.int64)
        f2 = np.zeros(PP_pad, np.int64)
        f1[pslot[m]] = r1[m]
        f2[pslot[m]] = r2[m]
        p.perm[r][pslot[m]] = porder[m]
        b1s, b2s = [], []
        for bkt in range(NBK):
            c0, nch = p.pcol[bkt], p.Pchp[bkt]
            if nch == 0:
                continue
            b1s.append(_wrap_idx(f1[c0 * 128:(c0 + nch) * 128]))
            b2s.append(_wrap_idx(f2[c0 * 128:(c0 + nch) * 128]))
        p.pidx1[r] = np.hstack(b1s)
        p.pidx2[r] = np.hstack(b2s)
    return p


def _build(p):
    """Build the SPMD Bass program for plan `p`."""
    nc = bacc.Bacc("TRN2", num_devices=R, num_swdge_queues=NQ)
    NBt, NBK = p.NB, p.NB * p.NB

    xtab_d = nc.dram_tensor("xtab", [p.TOT_ROWS, 128], bf16, kind="ExternalInput")
    S_d = nc.dram_tensor("Smat", [128, p.CT, 128], bf16, kind="ExternalInput")
    idx_d = nc.dram_tensor("idxE", [128, p.CT * 8], mybir.dt.int16, kind="ExternalInput")
    pi1_d = nc.dram_tensor("pidx1", [128, p.PCT * 8], mybir.dt.int16, kind="ExternalInput")
    pi2_d = nc.dram_tensor("pidx2", [128, p.PCT * 8], mybir.dt.int16, kind="ExternalInput")
    Ws_d = nc.dram_tensor("Ws", [65, 4, 64], bf16, kind="ExternalInput")
    wdbd_d = nc.dram_tensor("wdbd", [128, 65], f32, kind="ExternalInput")
    pout_d = nc.dram_tensor("pout", [128, p.PCT, 2], f32, kind="ExternalOutput")

    rg = [list(range(R))]
    qrr = [0]

    def next_q():
        q = (qrr[0] + 1) % NQ   # 1,2,3,0,... (q0 blocks the Pool queue)
        qrr[0] += 1
        return q

    with tile.TileContext(nc) as tc:
        with tc.tile_pool(name="dloc", bufs=1, space="DRAM") as dloc, \
             tc.tile_pool(name="sb", bufs=1) as sb, \
             tc.tile_pool(name="ps", bufs=1, space="PSUM") as ps:

            stage_dram = dloc.tile([128, p.TPR, 128], bf16)
            fulls = [dloc.tile([p.TOT_ROWS, 128], bf16, tag=f"full{i}",
                               name=f"full{i}", addr_space="Shared")
                     for i in range(3)]

            idx_t = sb.tile([128, p.CT * 8], mybir.dt.int16)
            pi1_t = sb.tile([128, p.PCT * 8], mybir.dt.int16)
            pi2_t = sb.tile([128, p.PCT * 8], mybir.dt.int16)
            Ws_t = sb.tile([65, 4, 64], bf16)
            wdbd_t = sb.tile([128, 65], f32)
            for t_, d_ in ((idx_t, idx_d), (pi1_t, pi1_d), (pi2_t, pi2_d),
                           (Ws_t, Ws_d), (wdbd_t, wdbd_d)):
                nc.sync.dma_start(out=t_[:], in_=d_[:])

            # two explicit acc buffers with a ones row at partition 64
            accs = [sb.tile([65, 128], bf16, tag=f"accsb{i}", name=f"accsb{i}")
                    for i in range(4)]
            for a in accs:
                nc.vector.memset(a[64:65, :], 1.0)

            stage_sb = sb.tile([128, p.TPR, 128], bf16)

            for l in range(3):
                table = xtab_d if l == 0 else fulls[l - 1]
                for g in range(p.NG):
                    ts = range(g * GT, min((g + 1) * GT, p.TPR))
                    Gs, Ss, c0s = {}, {}, {}
                    for (gg, b, c0, nch) in p.gathers:
                        if gg != g or nch == 0:
                            continue
                        c0s[b] = c0
                        Gt = sb.tile([128, nch, 128], bf16, tag="G", bufs=10)
                        lo = b * p.BSZ
                        hi = min(lo + p.BSZ, p.TOT_ROWS)
                        nc.gpsimd.dma_gather(
                            out_ap=Gt[:], in_ap=table[lo:hi, :],
                            idxs_ap=idx_t[:, c0 * 8:(c0 + nch) * 8],
                            num_idxs=nch * 128, num_idxs_reg=nch * 128,
                            elem_size=128, single_packet=False,
                            queue_num=next_q())
                        St = sb.tile([128, nch, 128], bf16, tag="S", bufs=10)
                        nc.scalar.dma_start(out=St[:], in_=S_d[:, c0:c0 + nch, :])
                        Gs[b], Ss[b] = Gt, St
                    g_lo, g_hi = g * GT, min((g + 1) * GT, p.TPR)
                    for t in ts:
                        nch_t = int(p.Pch[t, :].sum())
                        if nch_t == 0:
                            continue
                        acc = ps.tile([64, V], f32, tag="acc", space="PSUM", bufs=2)
                        ki = 0
                        for b in range(NBt):
                            base = int(p.col_run[t, b] - c0s.get(b, 0))
                            for k in range(int(p.Pch[t, b])):
                                nc.tensor.matmul(
                                    out=acc[:],
                                    lhsT=Gs[b][:, base + k, 0:64],
                                    rhs=Ss[b][:, base + k, :],
                                    start=(ki == 0), stop=(ki == nch_t - 1))
                                ki += 1
                        a_sb = accs[t % 4]
                        nc.vector.tensor_copy(a_sb[0:64, :], acc[:])
                        if l < 2:
                            hp = ps.tile([128, 64], f32, tag="hp", space="PSUM",
                                         bufs=6)
                            nc.tensor.matmul(out=hp[:], lhsT=a_sb[:, :],
                                             rhs=Ws_t[:, l, :], start=True,
                                             stop=True)
                            nc.vector.tensor_scalar(
                                out=stage_sb[:, t, 0:64], in0=hp[:],
                                scalar1=0.0, scalar2=None,
                                op0=mybir.AluOpType.max)
                        else:
                            up = ps.tile([128, 64], f32, tag="hp", space="PSUM",
                                         bufs=6)
                            vp = ps.tile([128, 64], f32, tag="hp", space="PSUM",
                                         bufs=6)
                            nc.tensor.matmul(out=up[:], lhsT=a_sb[:, :],
                                             rhs=Ws_t[:, 2, :], start=True,
                                             stop=True)
                            nc.tensor.matmul(out=vp[:], lhsT=a_sb[:, :],
                                             rhs=Ws_t[:, 3, :], start=True,
                                             stop=True)
                            nc.vector.tensor_copy(stage_sb[:, t, 0:64], up[:])
                            nc.vector.tensor_copy(stage_sb[:, t, 64:128], vp[:])
                    nc.sync.dma_start(out=stage_dram[:, g_lo:g_hi, :],
                                      in_=stage_sb[:, g_lo:g_hi, :])
                if l < 2:
                    nc.gpsimd.collective_compute(
                        "AllGather", mybir.AluOpType.bypass, replica_groups=rg,
                        ins=[stage_dram[:]], outs=[fulls[l][:]])

            # ---- pair stage ----
            # u-gathers read the local uv stage (stage_dram rows) and are
            # emitted BEFORE the final AllGather so they overlap it;
            # v-gathers read the gathered table per band afterwards.
            uvf = fulls[2]
            stage_rows = stage_dram.tensor.reshape([p.ROWS_PR, 128])
            SUBB = 12
            blocks = []
            for bkt in range(NBt):
                c0, nch = int(p.pcol[bkt]), int(p.Pchp[bkt])
                for s0 in range(0, nch, SUBB):
                    blocks.append((bkt, c0 + s0, min(SUBB, nch - s0)))
            NBLK = len(blocks)
            Uts = []
            for (bkt, c0, nch) in blocks:
                Ut = sb.tile([128, nch, 128], bf16, tag="U", bufs=NBLK)
                nc.gpsimd.dma_gather(
                    out_ap=Ut[:], in_ap=stage_rows[:, :],
                    idxs_ap=pi1_t[:, c0 * 8:(c0 + nch) * 8],
                    num_idxs=nch * 128, num_idxs_reg=nch * 128,
                    elem_size=128, single_packet=False,
                    queue_num=next_q())
                Uts.append(Ut)
            nc.gpsimd.collective_compute(
                "AllGather", mybir.AluOpType.bypass, replica_groups=rg,
                ins=[stage_dram[:]], outs=[fulls[2][:]])
            for (bkt, c0, nch), Ut in zip(blocks, Uts):
                Vt = sb.tile([128, nch, 128], bf16, tag="Vt", bufs=6)
                lo = bkt * p.BSZ
                hi = min(lo + p.BSZ, p.TOT_ROWS)
                nc.gpsimd.dma_gather(
                    out_ap=Vt[:], in_ap=uvf[lo:hi, :],
                    idxs_ap=pi2_t[:, c0 * 8:(c0 + nch) * 8],
                    num_idxs=nch * 128, num_idxs_reg=nch * 128,
                    elem_size=128, single_packet=False,
                    queue_num=next_q())
                z = sb.tile([128, nch, 64], f32, tag="z", bufs=2)
                nc.vector.tensor_tensor(out=z[:], in0=Ut[:, :, 0:64],
                                        in1=Vt[:, :, 64:128],
                                        op=mybir.AluOpType.add)
                nc.vector.tensor_scalar_max(z[:], z[:], 0.0)
                nc.vector.tensor_tensor(
                    out=z[:], in0=z[:],
                    in1=wdbd_t[:, 0:64].unsqueeze(1).to_broadcast([128, nch, 64]),
                    op=mybir.AluOpType.mult)
                ds = sb.tile([128, nch], f32, tag="ds", bufs=2)
                nc.vector.tensor_reduce(out=ds[:], in_=z[:],
                                        axis=mybir.AxisListType.X,
                                        op=mybir.AluOpType.add)
                po = sb.tile([128, nch, 2], f32, tag="po", bufs=2)
                nc.scalar.activation(po[:, :, 1:2], ds[:].unsqueeze(2),
                                     mybir.ActivationFunctionType.Sigmoid,
                                     bias=wdbd_t[:, 64:65], scale=1.0)
                nc.vector.tensor_scalar(
                    out=po[:, :, 0:1], in0=po[:, :, 1:2],
                    scalar1=-1.0, scalar2=1.0,
                    op0=mybir.AluOpType.mult, op1=mybir.AluOpType.add)
                nc.sync.dma_start(out=pout_d[:, c0:c0 + nch, :], in_=po[:])
    nc.compile()
    return nc


def _split_excess_waits(nc, max_waits=1):
    """Walrus rejects >1 sem wait on queue instructions; hoist extras onto
    standalone EventSemaphore instructions placed just before."""
    for fn in nc.m.functions:
        for bb in fn.blocks:
            il = bb.instructions
            new_list = []
            changed = False
            for ins in il:
                si = ins.sync_info
                if si is not None and si.on_wait and len(si.on_wait) > max_waits:
                    waits = list(si.on_wait)
                    keep, excess = waits[:max_waits], waits[max_waits:]
                    for gi in range(0, len(excess), max_waits):
                        ev = mybir.InstEventSemaphore(
                            name=f"{ins.name}_wsplit{gi}", ins=[], outs=[])
                        ev.engine = ins.engine
                        ev.sync_info = mybir.SyncInfo(
                            on_wait=excess[gi:gi + max_waits], on_update=[])
                        new_list.append(ev)
                    ins.sync_info = mybir.SyncInfo(
                        on_wait=keep, on_update=list(si.on_update))
                    changed = True
                new_list.append(ins)
            if changed:
                bb.instructions = new_list


def kernel(x, src, dst, gene1, gene2, W1, b1, W2, b2, W3, b3,
           Wfc1, bfc1, Wfc2, bfc2, _trace=False):
    x = np.asarray(x, np.float32)
    src = np.asarray(src, np.int64)
    dst = np.asarray(dst, np.int64)
    gene1 = np.asarray(gene1, np.int64)
    gene2 = np.asarray(gene2, np.int64)
    W1, b1 = np.asarray(W1, np.float32), np.asarray(b1, np.float32)
    W2, b2 = np.asarray(W2, np.float32), np.asarray(b2, np.float32)
    W3, b3 = np.asarray(W3, np.float32), np.asarray(b3, np.float32)
    Wfc1, bfc1 = np.asarray(Wfc1, np.float32), np.asarray(bfc1, np.float32)
    Wfc2, bfc2 = np.asarray(Wfc2, np.float32), np.asarray(bfc2, np.float32)

    p = _make_plan(x, src, dst, gene1, gene2)

    # host-folded constants: Ws rows 0:64 = weight, row 64 = bias
    Mu = W3 @ Wfc1[:64]
    Mv = W3 @ Wfc1[64:]
    bu = b3 @ Wfc1[:64] + 0.5 * bfc1
    bv = b3 @ Wfc1[64:] + 0.5 * bfc1
    Ws = np.zeros((65, 4, 64), np.float32)
    Ws[:64, 0], Ws[64, 0] = W1, b1
    Ws[:64, 1], Ws[64, 1] = W2, b2
    Ws[:64, 2], Ws[64, 2] = Mu, bu
    Ws[:64, 3], Ws[64, 3] = Mv, bv
    wdiff = (Wfc2[:, 1] - Wfc2[:, 0]).astype(np.float32)
    bd = float(bfc2[1] - bfc2[0])
    wdbd = np.zeros((128, 65), np.float32)
    wdbd[:, 0:64] = wdiff[None, :]
    wdbd[:, 64] = bd

    nc = _build(p)
    if not os.environ.get("GCN_SIM"):
        _split_excess_waits(nc)

    # replicated x table (layer-1 gather source), bf16, baseline row layout
    xtab = np.zeros((p.TOT_ROWS, 128), _BF)
    xb = x.astype(_BF)
    for r in range(R):
        lo = r * p.NPR
        hi = min(lo + p.NPR, p.N)
        blk = np.zeros((p.ROWS_PR, 64), _BF)
        blk[:hi - lo] = xb[lo:hi]
        # local node l -> row TPR*(l%128) + l//128
        blk = blk.reshape(p.TPR, 128, 64).transpose(1, 0, 2).reshape(p.ROWS_PR, 64)
        xtab[r * p.ROWS_PR:(r + 1) * p.ROWS_PR, 0:64] = blk

    in_maps = []
    for r in range(R):
        m = {
            "xtab": xtab,
            "Smat": p.S2[r].reshape(128, p.CT, 128),
            "idxE": p.idx2[r],
            "pidx1": p.pidx1[r], "pidx2": p.pidx2[r],
            "Ws": Ws.astype(_BF), "wdbd": wdbd,
        }
        in_maps.append(m)

    if os.environ.get("GCN_SIM"):
        from concourse.bass_interp import MultiCoreSim
        sim = MultiCoreSim(nc, R)
        for r in range(R):
            for k, v in in_maps[r].items():
                sim.cores[r].tensor(k)[:] = v
        sim.simulate()
        results = [{"pout": np.asarray(sim.cores[rr].mem_tensor("pout"))
                    .reshape(128, p.PCT, 2)} for rr in range(R)]

        class _R:
            pass
        res = _R()
        res.results = results
    else:
        res = run_bass_kernel_spmd(nc, in_maps, core_ids=list(range(R)),
                                   trace=_trace)

    out = np.zeros((p.NP, 2), np.float32)
    for r in range(R):
        po = np.asarray(res.results[r]["pout"]).reshape(128, p.PCT, 2)
        flat = po.transpose(1, 0, 2).reshape(-1, 2)   # slot j = c*128 + p
        valid = p.perm[r] >= 0
        out[p.perm[r][valid]] = flat[valid]
    if _trace:
        kernel.last_results = res
    return out
